# revision 14
# baseline (speedup 1.0000x reference)
"""Trainium2 Bass kernel for nn_Additive (all-pairs additive-attention scorer).

Computes, for x [N, D], y [M, D], W1 [2D, D], W2 [D, 1]:
    hx = x @ W1[:D];  hy = y @ W1[D:]
    out[i, j] = sum_d relu(hx[i, d] + hy[j, d]) * W2[d]      -> [N, M, 1]

Strategy (8 NeuronCores, data-parallel over rows of x):
  - Each core owns a 128-row block of x; y / W1 / W2 are replicated.
  - Device layout puts the hidden dim d on SBUF partitions (4 chunks of 128).
  - z = relu(hyT_chunk + hxT_chunk[:, i]) is one fused instruction per
    (i, chunk): ACT `activation(Relu, bias=per-partition)` or DVE
    `tensor_scalar(add, max)` — both engines run concurrently on different i.
    hyT/z are fp16 (DVE 4x mode; bias and accumulation stay fp32 —
    rel-to-scale error ~3e-4 vs the fp32 reference).
  - The contraction over d with W2 rides the TensorEngine: W2 chunk is the
    stationary operand, z streams as the moving operand (N=512), with 4 i's
    in flight concurrently via col-group tile_position packing.
  - One persistent PSUM tile [128, 1024]: strip t's partition 32t+r holds
    the j-row for i = 4r+t. Groups run r descending; a group's matmuls
    write rows 32t..32t+r (rows below r are garbage that later, lower-r
    groups overwrite; row r is final). One copy + one permuted DMA at the
    end instead of per-group evictions.
"""

import numpy as np

DIM = 512
N_ROWS = 1024
M_ROWS = 1024
N_CORES = 8
IPC = N_ROWS // N_CORES  # 128 rows of x per core
GRP = 4                  # i's processed concurrently (PE col strips)
NGRP = IPC // GRP        # 32 groups per core
KC = DIM // 128          # 4 partition chunks of the hidden dim

_cached = {}


def _patch_tile_drain():
    """The walrus build in this container caps sem-waits per CTRL
    instruction; stock TileContext piles every proc's final-value wait onto
    one tail drain. Split them one-per-instruction."""
    import concourse.mybir as mybir
    from concourse.tile import TileContext
    from concourse.vector_clock import ScopedClock

    if getattr(TileContext, "_drain_split_patch", False):
        return

    def _drain_and_barrier(self, tick_clock, wait_clock):
        nc = self.nc
        drain_inst = nc.sync.drain()
        wait_clock.add_sem_waits(
            drain_inst.ins, ScopedClock({None: tick_clock.global_clock})
        )
        mi = drain_inst.ins
        si = mi.sync_info
        if si is not None and si.on_wait and len(si.on_wait) > 1:
            waits = list(si.on_wait)
            mi.sync_info = mybir.SyncInfo(
                on_wait=[waits[0]], on_update=list(si.on_update)
            )
            for w in waits[1:]:
                nop = nc.sync.nop(nofuse=True)
                nop.ins.sync_info = mybir.SyncInfo(on_wait=[w], on_update=[])
        nc.all_engine_barrier()
        popped = nc._tile_sem_poison_stack.pop()
        assert popped is self._sem_poison
        nc.clear_and_free_semaphores(list(self.sems.allocated().values()))
        nc.all_engine_barrier()

    TileContext._drain_and_barrier = _drain_and_barrier
    TileContext._drain_split_patch = True


def _split_sync_waits(nc, cap=1):
    """Walrus in this container caps sync-wait commands per instruction.
    Hoist excess waits onto same-engine NoOps inserted just before the
    over-limit instruction (engine executes in order, so semantics hold)."""
    import concourse.mybir as mybir

    for bb in nc.main_func.blocks:
        pending = []  # (index, nop_inst)
        for idx, ins in enumerate(bb.instructions):
            si = ins.sync_info
            if si is None or not si.on_wait or len(si.on_wait) <= cap:
                continue
            waits = list(si.on_wait)
            keep, extra = waits[:cap], waits[cap:]
            ins.sync_info = mybir.SyncInfo(
                on_wait=keep, on_update=list(si.on_update)
            )
            for off in range(0, len(extra), cap):
                chunk = extra[off : off + cap]
                nop = mybir.InstNoOp(
                    name=nc.get_next_instruction_name(),
                    engine=ins.engine,
                    bass_nofuse=True,
                    ins=[],
                    outs=[],
                    sync_info=mybir.SyncInfo(on_wait=chunk, on_update=[]),
                )
                nc.register_instruction(nop, overwrite=True)
                pending.append((idx, nop))
        for idx, nop in reversed(pending):
            bb.instructions.insert(idx, nop)


# engine for each z op, indexed by (c * GRP + t) % 16.
# "v" = DVE tensor_scalar, "a" = ACT activation, "g" = GPSIMD tensor_scalar
Z_ENGINES = "vvvavvvavvvavvva"


def _build_program():
    import concourse.bass as bass
    import concourse.mybir as mybir
    from concourse.tile import TileContext

    _patch_tile_drain()
    f32 = mybir.dt.float32
    f16 = mybir.dt.float16
    Alu = mybir.AluOpType
    Act = mybir.ActivationFunctionType

    nc = bass.Bass()
    xT_d = nc.declare_dram_parameter("xT", [DIM, IPC], f16, isOutput=False)
    yT_d = nc.declare_dram_parameter("yT", [DIM, M_ROWS], f16, isOutput=False)
    w1_d = nc.declare_dram_parameter("w1", [2 * DIM, DIM], f16, isOutput=False)
    w2_d = nc.declare_dram_parameter("w2b", [128, KC * 32], f16, isOutput=False)
    out_d = nc.declare_dram_parameter("out_block", [IPC, M_ROWS], f32, isOutput=True)

    with TileContext(nc) as tc:
        with tc.tile_pool(name="const", bufs=1) as cpool:
            w2_sb = cpool.tile([128, KC * 32], f16, name="w2_sb", tag="w2")
            nc.sync.dma_start(out=w2_sb[:], in_=w2_d[:])
            hxT = [
                cpool.tile([128, IPC], f32, name=f"hxT{m}", tag=f"hxT{m}")
                for m in range(KC)
            ]
            hyT = [
                cpool.tile([128, M_ROWS], f16, name=f"hyT{m}", tag=f"hyT{m}")
                for m in range(KC)
            ]

            # ---- preamble: hxT = (x @ W1x)^T, hyT = (y @ W1y)^T ----
            # input DMAs spread across engines so several HWDGE queues pull
            # from HBM concurrently; y first (it gates every hyT chunk).
            with (
                tc.tile_pool(name="pre", bufs=1) as pre,
                tc.tile_pool(name="prepsum", bufs=2, space="PSUM") as ppool,
            ):
                dma_engines = [nc.sync, nc.scalar]
                # x and W1x land first (they gate hxT, which every z needs)
                xT_sb = []
                for k in range(KC):
                    t = pre.tile([128, IPC], f16, name=f"xT{k}", tag=f"xT{k}")
                    dma_engines[k % 2].dma_start(
                        out=t[:], in_=xT_d[128 * k : 128 * (k + 1), :]
                    )
                    xT_sb.append(t)
                w1_sb = [None] * (2 * KC)
                for j, m in enumerate(list(range(KC)) + list(range(KC, 2 * KC))):
                    t = pre.tile([128, DIM], f16, name=f"w1_{m}", tag=f"w1_{m}")
                    dma_engines[j % 2].dma_start(
                        out=t[:], in_=w1_d[128 * m : 128 * (m + 1), :]
                    )
                    w1_sb[m] = t
                yT_sb = []
                for k in range(KC):
                    t = pre.tile([128, M_ROWS], f16, name=f"yT{k}", tag=f"yT{k}")
                    dma_engines[k % 2].dma_start(
                        out=t[:], in_=yT_d[128 * k : 128 * (k + 1), :]
                    )
                    yT_sb.append(t)

                # 4 concurrent col strips (M=32) so the K-accumulation
                # chains overlap on the PE instead of serializing
                for m in range(KC):
                    ps = ppool.tile([128, IPC], f32, name="psx", tag="psx")
                    for k in range(KC):
                        for s in range(4):
                            nc.tensor.matmul(
                                ps[32 * s : 32 * (s + 1), :],
                                w1_sb[k][:, 128 * m + 32 * s : 128 * m + 32 * (s + 1)],
                                xT_sb[k][:],
                                start=(k == 0),
                                stop=(k == KC - 1),
                                tile_position=(0, 32 * s),
                            )
                    nc.vector.tensor_copy(hxT[m][:], ps[:])
                for m in range(KC):
                    for jh in range(2):
                        ps = ppool.tile([128, 512], f32, name="psy", tag="psy")
                        for k in range(KC):
                            for s in range(4):
                                nc.tensor.matmul(
                                    ps[32 * s : 32 * (s + 1), :],
                                    w1_sb[KC + k][
                                        :, 128 * m + 32 * s : 128 * m + 32 * (s + 1)
                                    ],
                                    yT_sb[k][:, 512 * jh : 512 * (jh + 1)],
                                    start=(k == 0),
                                    stop=(k == KC - 1),
                                    tile_position=(0, 32 * s),
                                )
                        if jh == 0:
                            nc.vector.tensor_copy(
                                hyT[m][:, 512 * jh : 512 * (jh + 1)], ps[:]
                            )
                        else:
                            nc.scalar.copy(
                                hyT[m][:, 512 * jh : 512 * (jh + 1)], ps[:]
                            )

            # ---- main loop ----
            with (
                tc.tile_pool(name="z", bufs=8) as zpool,
                tc.tile_pool(name="ops", bufs=1, space="PSUM") as opool,
                tc.tile_pool(name="osb", bufs=1) as osb_pool,
            ):
                ps = opool.tile([128, M_ROWS], f32, name="ops", tag="out")
                for g in range(NGRP):
                    r = NGRP - 1 - g
                    for c in range(KC):
                        for t in range(GRP):
                            i = GRP * r + t
                            z = zpool.tile([128, M_ROWS], f16, name="z", tag="z")
                            eng = Z_ENGINES[(c * GRP + t) % 16]
                            if eng == "a":
                                nc.scalar.activation(
                                    z[:], hyT[c][:], Act.Relu,
                                    bias=hxT[c][:, i : i + 1],
                                )
                            elif eng == "g":
                                nc.gpsimd.tensor_scalar(
                                    out=z[:], in0=hyT[c][:],
                                    scalar1=hxT[c][:, i : i + 1],
                                    scalar2=0.0,
                                    op0=Alu.add, op1=Alu.max,
                                )
                            else:
                                nc.vector.tensor_scalar(
                                    out=z[:], in0=hyT[c][:],
                                    scalar1=hxT[c][:, i : i + 1],
                                    scalar2=0.0,
                                    op0=Alu.add, op1=Alu.max,
                                )
                            for jh in range(2):
                                nc.tensor.matmul(
                                    ps[32 * t : 32 * t + r + 1,
                                       512 * jh : 512 * (jh + 1)],
                                    w2_sb[:, 32 * c : 32 * c + r + 1],
                                    z[:, 512 * jh : 512 * (jh + 1)],
                                    start=(c == 0),
                                    stop=(c == KC - 1),
                                    tile_position=(0, 32 * t),
                                )
                out_sb = osb_pool.tile([128, M_ROWS], f32, name="out_sb", tag="osb")
                nc.vector.tensor_copy(out_sb[:, 0:512], ps[:, 0:512])
                nc.scalar.copy(out_sb[:, 512:1024], ps[:, 512:1024])
                # partition p = 32t + s holds row i = 4s + t; the host
                # undoes that permutation, so this DMA is contiguous.
                nc.sync.dma_start(out=out_d[:], in_=out_sb[:])
    _split_sync_waits(nc)
    return nc


def _get_program():
    if "nc" not in _cached:
        _cached["nc"] = _build_program()
    return _cached["nc"]


def kernel(x, y, W1, W2, is_pairwise=0, **_unused):
    x = np.asarray(x, dtype=np.float32)
    y = np.asarray(y, dtype=np.float32)
    W1 = np.asarray(W1, dtype=np.float32)
    W2 = np.asarray(W2, dtype=np.float32)

    if int(np.asarray(is_pairwise)) != 0:
        # Not exercised by this problem (is_pairwise is always 0); tiny
        # N*D fallback kept for contract completeness.
        d = x.shape[-1]
        h = np.maximum(x @ W1[:d] + y @ W1[d:], 0.0)
        return (h @ W2).astype(np.float32)

    from concourse.bass_utils import run_bass_kernel_spmd

    nc = _get_program()
    yT = np.ascontiguousarray(y.T.astype(np.float16))
    w1h = W1.astype(np.float16)
    w2flat = W2.reshape(-1)[:DIM]
    w2b = np.empty((128, KC * 32), dtype=np.float16)
    for c in range(KC):
        w2b[:, 32 * c : 32 * (c + 1)] = (
            w2flat[128 * c : 128 * (c + 1)].astype(np.float16)[:, None]
        )
    in_maps = []
    for c in range(N_CORES):
        xT_c = np.ascontiguousarray(x[c * IPC : (c + 1) * IPC, :].T.astype(np.float16))
        in_maps.append({"xT": xT_c, "yT": yT, "w1": w1h, "w2b": w2b})

    res = run_bass_kernel_spmd(nc, in_maps, list(range(N_CORES)))
    # device row p = 32t + s holds output row i = 4s + t of the block
    perm = np.array([32 * (i % GRP) + i // GRP for i in range(IPC)])
    out = np.concatenate(
        [res.results[c]["out_block"][perm] for c in range(N_CORES)], axis=0
    )
    return out[:, :, None].astype(np.float32)
